# revision 1
# baseline (speedup 1.0000x reference)
"""Trainium2 Bass kernel for nn_RasterPoints.

reference semantics:
    idx = (x.reshape(B,T,P,2) / resolution[:,:,None,:] + origin[:,:,None,:]).astype(int32)
    out = zeros(B,T,H,W,P); out[b,t,idx[...,1],idx[...,0],p] = 1.0

Strategy (scatter_memory, memory regime):
  - Shard batch B=16 across 8 cores (2 batches/core -> 100 MB output/core).
  - The output is zeros + 2500 ones per core. On-device work is:
      (1) bulk zero-fill via big SBUF->HBM DMAs from a memset zero buffer
      (2) indirect-DMA scatter of ones at precomputed int32 element offsets
  - Index math is done host-side in fp32 numpy — bit-identical to the jax
    reference (IEEE div/add + trunc toward zero), avoiding any HW divide
    mismatch risk.
  - The per-core output is split into 13 DRAM chunk tensors so each chunk's
    scatter only depends on that chunk's zero-fill (pipelines cleanly).
"""

import numpy as np

from concourse import bass, mybir
import concourse.tile as tile
from concourse.bass_utils import run_bass_kernel_spmd

# Problem shape (hardcoded per contract)
B, T, P2 = 16, 50, 50
P = P2 // 2            # 25 points
H, W = 100, 100
NCORES = 8
B_PER = B // NCORES    # 2 batches per core
SLABS = B_PER * T      # 100 slabs per core
SLAB = H * W * P       # 250000 f32 = 1 MB per slab

# Chunking: 12 chunks of 8 slabs (8 MB) + 1 chunk of 4 slabs (4 MB).
# 8 slabs = 2M f32 = [128, 15625] view (128 partitions -> all 16 SDMA engines
# at full width); measured fastest among 4/8/16-slab chunkings.
DEFAULT_CHUNK_SLABS = (8,) * 12 + (4,)
IDX_PER_CALL = 100     # one scatter call covers 4 slabs' worth of points
PAD_IDX = np.int32(2**30)      # > bounds_check -> silently dropped by HW


def _chunk_meta(chunk_slabs):
    chunk_slabs = list(chunk_slabs)
    assert sum(chunk_slabs) == SLABS
    starts = np.cumsum([0] + chunk_slabs[:-1]).tolist()
    calls = [cs * P // IDX_PER_CALL for cs in chunk_slabs]
    assert all(cs * P % IDX_PER_CALL == 0 for cs in chunk_slabs)
    ncalls = sum(calls)

    zf = max((cs * SLAB) // 128 for cs in chunk_slabs)
    return chunk_slabs, starts, calls, ncalls, zf


def _split_big_waits(nc, maxw=1):
    """This walrus build rejects >maxw sem-waits on one instruction (the
    Tile tail drain carries several). Offload excess waits onto NoOps."""
    for bb in nc.main_func.blocks:
        new_list = []
        for ins in bb.instructions:
            si = ins.sync_info
            if si is not None and si.on_wait is not None and len(si.on_wait) > maxw:
                waits = list(si.on_wait)
                carriers = waits[:-maxw]
                keep = waits[len(carriers):]
                for j, w in enumerate(carriers):
                    nop = mybir.InstNoOp(name=f"{ins.name}-wsplit{j}", ins=[], outs=[])
                    nop.engine = ins.engine
                    nop.sync_info = mybir.SyncInfo(on_wait=[w], on_update=[])
                    new_list.append(nop)
                si.on_wait = keep
            new_list.append(ins)
        bb.instructions[:] = new_list


_CACHED_NC = {}


def _build_program(
    reps=1,
    skip_scatter=False,
    zero_engines=("sync", "scalar"),
    chunk_slabs=DEFAULT_CHUNK_SLABS,
    zero_splits=1,
    max_last=None,
):
    """Build the SPMD program. reps>1 repeats the whole device body (used
    only by timing harnesses to measure per-rep HW time via slope).
    skip_scatter / zero_engines / chunk_slabs / zero_splits are knobs."""
    key = (
        reps,
        skip_scatter,
        tuple(zero_engines),
        tuple(chunk_slabs),
        zero_splits,
        max_last,
    )
    if key in _CACHED_NC:
        return _CACHED_NC[key]

    chunk_slabs, chunk_starts, calls_per_chunk, ncalls, zf = _chunk_meta(chunk_slabs)
    nchunk = len(chunk_slabs)

    nc = bass.Bass()
    idx_in = nc.declare_dram_parameter(
        "idx", [IDX_PER_CALL, ncalls], mybir.dt.int32, isOutput=False
    )
    chunks = [
        nc.declare_dram_parameter(
            f"out{c}", [chunk_slabs[c] * SLAB, 1], mybir.dt.float32, isOutput=True
        )
        for c in range(nchunk)
    ]

    with tile.TileContext(nc) as tc:
        with tc.tile_pool(name="sbuf", bufs=1) as pool:
            zbuf = pool.tile([128, zf], mybir.dt.float32)
            # split the memset across both memset-capable engines so the
            # first zero-DMA can start sooner
            half = zf // 2
            nc.vector.memset(zbuf[:, 0:half], 0.0)
            nc.gpsimd.memset(zbuf[:, half:zf], 0.0)

            ones = pool.tile([IDX_PER_CALL, 1], mybir.dt.float32)
            nc.vector.memset(ones[:], 1.0)
            idx_all = pool.tile([IDX_PER_CALL, ncalls], mybir.dt.int32)
            nc.sync.dma_start(out=idx_all[:], in_=idx_in[:])

            # one bounds register per distinct chunk size (indirect_dma_start
            # would otherwise allocate a fresh gpsimd register per call)
            bounds_regs = {
                n: nc.gpsimd.to_reg(n * SLAB - 1) for n in sorted(set(chunk_slabs))
            }

            zeng_i = 0
            for _rep in range(reps):
                call = 0
                for c in range(nchunk):
                    n_elem = chunk_slabs[c] * SLAB
                    # zero-fill with a [128, n/128] view — 128 partitions is
                    # the only shape that runs all 16 SDMA engines at full
                    # rate (measured: 125- or 100-partition views run at a
                    # fraction of the bandwidth). A non-divisible chunk gets
                    # a main [128, n//128] DMA plus a tiny <=127-element
                    # remainder DMA.
                    main = (n_elem // 128) * 128
                    fdim = main // 128
                    assert fdim <= zf
                    zview = chunks[c][0:main, :].rearrange(
                        "(a b) o -> a (b o)", a=128
                    )
                    splits = zero_splits if fdim % zero_splits == 0 else 1
                    fs = fdim // splits
                    for s in range(splits):
                        zeng = getattr(nc, zero_engines[zeng_i % len(zero_engines)])
                        zeng_i += 1
                        zeng.dma_start(
                            out=zview[:, s * fs : (s + 1) * fs],
                            in_=zbuf[:128, s * fs : (s + 1) * fs],
                            max_dma_last_dim=max_last,
                        )
                    rem = n_elem - main
                    if rem:
                        zeng = getattr(nc, zero_engines[zeng_i % len(zero_engines)])
                        zeng_i += 1
                        zeng.dma_start(
                            out=chunks[c][main:n_elem, :], in_=zbuf[:rem, :1]
                        )

                    if skip_scatter:
                        call += calls_per_chunk[c]
                        continue
                    for _k in range(calls_per_chunk[c]):
                        nc.gpsimd.indirect_dma_start(
                            out=chunks[c][:],
                            out_offset=bass.IndirectOffsetOnAxis(
                                ap=idx_all[:, call : call + 1], axis=0
                            ),
                            in_=ones[:, :1],
                            in_offset=None,
                            bounds_check=bounds_regs[chunk_slabs[c]],
                            oob_is_err=False,
                        )
                        call += 1

    _split_big_waits(nc, maxw=1)
    _CACHED_NC[key] = nc
    return nc


def _host_indices(x, resolution, origin, chunk_slabs=DEFAULT_CHUNK_SLABS):
    """Exact replica of the reference index math in numpy fp32.
    Returns per-core idx arrays [IDX_PER_CALL, ncalls] int32 (chunk-relative
    element offsets, padded with PAD_IDX for out-of-bounds points)."""
    chunk_slabs, chunk_starts, calls_per_chunk, ncalls, _zf = _chunk_meta(chunk_slabs)
    nchunk = len(chunk_slabs)
    x = np.asarray(x, dtype=np.float32)
    resolution = np.asarray(resolution, dtype=np.float32)
    origin = np.asarray(origin, dtype=np.float32)
    pts = x.reshape(B, T, P, 2)
    idx = (pts / resolution[:, :, None, :] + origin[:, :, None, :]).astype(np.int32)
    col = idx[..., 0].astype(np.int64)  # [B,T,P]
    row = idx[..., 1].astype(np.int64)
    valid = (row >= 0) & (row < H) & (col >= 0) & (col < W)

    per_core = []
    t_ar = np.arange(T)[None, :, None]
    p_ar = np.arange(P)[None, None, :]
    for core in range(NCORES):
        b0 = core * B_PER
        b_loc = np.arange(B_PER)[:, None, None]
        slab = b_loc * T + t_ar  # [B_PER,T,1]
        elem = (
            slab * SLAB
            + row[b0 : b0 + B_PER] * (W * P)
            + col[b0 : b0 + B_PER] * P
            + p_ar
        )  # [B_PER,T,P]
        v = valid[b0 : b0 + B_PER]
        slab_f = np.broadcast_to(slab, elem.shape).reshape(-1)
        elem_f = elem.reshape(-1)
        v_f = v.reshape(-1)

        arr = np.full((ncalls, IDX_PER_CALL), PAD_IDX, dtype=np.int32)
        call = 0
        for c in range(nchunk):
            s0 = chunk_starts[c]
            s1 = s0 + chunk_slabs[c]
            sel = (slab_f >= s0) & (slab_f < s1)
            rel = elem_f[sel] - s0 * SLAB
            vv = v_f[sel]
            rel = np.where(vv, rel, np.int64(PAD_IDX)).astype(np.int32)
            ncall = calls_per_chunk[c]
            assert rel.size == ncall * IDX_PER_CALL
            arr[call : call + ncall] = rel.reshape(ncall, IDX_PER_CALL)
            call += ncall
        per_core.append(np.ascontiguousarray(arr.T))  # [IDX_PER_CALL, ncalls]
    return per_core


def kernel(x, resolution, origin):
    nc = _build_program()
    idx_per_core = _host_indices(x, resolution, origin)
    in_maps = [{"idx": idx_per_core[c]} for c in range(NCORES)]
    res = run_bass_kernel_spmd(nc, in_maps, list(range(NCORES)))

    chunk_slabs, _starts, _calls, _ncalls, _zf = _chunk_meta(DEFAULT_CHUNK_SLABS)
    out = np.empty((B, T, H, W, P), dtype=np.float32)
    out_flat = out.reshape(NCORES, SLABS * SLAB)
    for core in range(NCORES):
        pos = 0
        for c in range(len(chunk_slabs)):
            n_elem = chunk_slabs[c] * SLAB
            out_flat[core, pos : pos + n_elem] = res.results[core][f"out{c}"].reshape(-1)
            pos += n_elem
    return out

